# revision 27
# baseline (speedup 1.0000x reference)
"""FFTEmbedding kernel for Trainium2 (8 NeuronCores, SPMD data-parallel over B).

Math: the reference computes, per (b, t):
    window = x_pad[b, t : t+W]                (causal window, W=256)
    spec   = rfft(window); feats = [spec.real, spec.imag]   (258)
    out    = feats @ weight.T + bias          (512)

The pipeline is linear in x, so it collapses to a causal 1-D convolution
with a precomputed (W=256, EMB=512) matrix:
    M2[w, e] = sum_k weight[e, k]*cos(2*pi*k*w/W) - weight[e, 129+k]*sin(2*pi*k*w/W)
    out[b, t, e] = sum_w x_pad[b, t+w] * M2[w, e] + bias[e]

Device mapping (per core: 2 batch rows, weights replicated). The kernel
is bound by the DMA fabric (16 SDMA engines shared by both HWDGE rings,
~25 GB/s each while streaming; every dma_start pays ~0.6-2 us of fixed
latency dominated by the completion receipt), so the design minimizes
DMA count, maximizes run length, and keeps the critical path alone on
one ring:
  * "mega-Hankel" SBUF image Hank[p, c] = x_pad[b, p+c] via overlapping
    DMA reads (partition p reads x_pad[b, p : p+...]); a 128-col slice
    IS the pre-transposed matmul lhsT. Batch 0 loads as 3 chunks on the
    sync ring ONLY (strict FIFO => the first chunk completes first);
    batch 1 loads as ONE 2.1 MB DMA on the scalar ring mid-stream.
  * per 128-t output tile i: psum[128, 512] = Hank[:, 128i:+128].T @ W0
    + Hank[:, 128(i+1):+128].T @ W1   (fp16 operands, fp32 PSUM accum).
    Two tiles share one 2-bank PSUM pair; evacuation is a single pure
    copy (psum fp32 -> SBUF fp16) alternating ACT / DVE, keeping both
    engines at ~65% so the PE never stalls on PSUM reuse even when the
    chip drops to its 2.0 GHz power state (runs vary 2.0 vs 2.4 GHz!).
    The bias is added on the host during the mandatory fp16 -> fp32
    upcast -- fused-bias evacuation variants saturated ACT/DVE, and
    gpsimd is unusable here (cannot read PSUM; its tensor ops measured
    ~5 us per pair AND hold the shared SBUF port DVE 2x ops need).
  * output uses a PARTITION-MAJOR DRAM layout out_dev[b, p, c*512+e] =
    out[b, 128c+p, e]: each 8-tile supertile store is a plain 2D slice
    with an 8 KB contiguous run per partition (measured 25.2 GB/s per
    packet vs 21.4 for the 1 KB runs of the natural layout). The host
    un-permutes with a cheap numpy reshape. Stores alternate rings; the
    final store is split across both rings to halve the drain.
  * PE warm-up: HAM clock gate needs ~3.4 us of sustained PE activity;
    N=128 junk matmuls bridge the input-DMA wait so the real stream
    starts at the warm clock rate.
  * output staged fp16 in SBUF, stored fp16 (halves the dominant HBM
    write stream); host upcasts to fp32. End-to-end rel err ~4e-4.
"""

import os
import sys

import numpy as np

_TRN_REPO = "/opt/trn_rl_repo"
if _TRN_REPO not in sys.path:
    sys.path.insert(0, _TRN_REPO)

B, T, W_SIZE, EMB = 16, 8192, 256, 512
N_CORES = 8
B_PER = B // N_CORES          # 2 batch rows per core
PAD = W_SIZE - 1              # 255 leading zeros
XP_LEN = T + PAD + 1          # 8448 (one trailing pad elem)
HANK_COLS = T + W_SIZE - 128  # 8320 mega-Hankel free dim
N_TILES = T // 128            # 64 output tiles of 128 t's per batch row
N_PAIRS = N_TILES // 2        # 32 PSUM pairs per batch row
PAIRS_PER_SUP = 4             # supertile = 4 pairs = 8 tiles = 1024 t rows
N_SUP = N_PAIRS // PAIRS_PER_SUP
SUP_W = 2 * PAIRS_PER_SUP * EMB  # 4096 fp16 cols = 8 KB/partition

CHUNKS = [512, 1024, 2560, 4224]  # batch-0 ladder, sum = 8320, runs 1-8.3 KB
OFFS = [sum(CHUNKS[:j]) for j in range(len(CHUNKS) + 1)]
N_JUNK = 44                   # N=128 warm-up matmuls bridging the DMA wait
                              # (~4.7us of PE activity: HAM un-throttles
                              # before the first real matmul, and the PE
                              # stays busy right up to the input gate
                              # (~12.4us fast / ~13.6us slow clock) instead
                              # of idling the last ~0.8us)

# module-level knobs (test.py pokes these)
TRACE = os.environ.get("KERNEL_TRACE", "0") == "1"
USE_DT = os.environ.get("KERNEL_DT", "fp16")      # matmul operand dtype
OUT_DT = os.environ.get("KERNEL_OUT_DT", "fp16")  # device output dtype
LAST_RESULT = None

_CACHE = {}


def _build_m2(weight: np.ndarray) -> np.ndarray:
    """(EMB, 258) projection -> (W, EMB) causal-conv matrix, in float64."""
    k = np.arange(W_SIZE // 2 + 1, dtype=np.float64)   # 129
    w = np.arange(W_SIZE, dtype=np.float64)            # 256
    ang = 2.0 * np.pi * np.outer(k, w) / W_SIZE        # (129, 256)
    f = np.concatenate([np.cos(ang), -np.sin(ang)], axis=0)  # (258, 256)
    m2 = (weight.astype(np.float64) @ f).T             # (256, EMB)
    return np.ascontiguousarray(m2, dtype=np.float64)


def _round_fp22(a: np.ndarray) -> np.ndarray:
    """Round fp32 -> fp22 (e8m13, the TensorE f32r operand precision)."""
    u = np.ascontiguousarray(a, dtype=np.float32).view(np.uint32)
    u = (u + np.uint32(0x200)) & np.uint32(0xFFFFFC00)
    return u.view(np.float32)


def _build_program():
    from concourse import bacc, mybir, tile
    from concourse.ap import AP

    f32 = mybir.dt.float32
    fin = {
        "fp16": mybir.dt.float16,
        "bf16": mybir.dt.bfloat16,
        "f32r": mybir.dt.float32r,
        "f32": f32,
    }[USE_DT]
    fout = {"fp16": mybir.dt.float16, "bf16": mybir.dt.bfloat16, "f32": f32}[OUT_DT]

    nc = bacc.Bacc(target_bir_lowering=False)
    xpad_h = nc.declare_dram_parameter("xpad", [B_PER, XP_LEN], fin, isOutput=False)
    # w2 pre-packed on host to the SBUF layout: w2[p, h*EMB+e] = M2[128h+p, e]
    w2_h = nc.declare_dram_parameter("w2", [128, 2 * EMB], fin, isOutput=False)
    # partition-major output: out_dev[b, p, c*EMB+e] = out[b, 128c+p, e]
    out_h = nc.declare_dram_parameter(
        "out", [B_PER, 128, N_TILES * EMB], fout, isOutput=True
    )

    with tile.TileContext(nc) as tc:
        with (
            tc.tile_pool(name="hank", bufs=1) as hank_pool,
            tc.tile_pool(name="wpool", bufs=1) as w_pool,
            tc.tile_pool(name="cpool", bufs=1) as c_pool,
            tc.tile_pool(name="sup", bufs=5) as sup_pool,
            tc.tile_pool(name="psum", bufs=4, space="PSUM") as psum_pool,
        ):
            # PE pre-warm: N=128 junk matmuls keep the PE busy from engine
            # release, lifting the HAM 1.2 GHz cold throttle while the
            # input DMAs are in flight.
            junk = c_pool.tile([128, 128], fin, tag="junk")
            nc.gpsimd.memset(junk[:, :], 0.0)
            ps_warm = psum_pool.tile([128, 2 * EMB], f32, tag="ps")
            for _ in range(N_JUNK):
                nc.tensor.matmul(
                    ps_warm[:, 0:128], junk[:, :], junk[:, :],
                    start=True, stop=True,
                )

            # critical path rides the sync ring ALONE, strict FIFO, in
            # first-need order; every completion semaphore carries ~2us of
            # write-receipt latency, so the first pieces are kept small:
            # w0 -> hank c0 (512 cols) -> w1 -> c1 -> c2 -> c3 -> hk1
            w01 = w_pool.tile([128, 2 * EMB], fin, tag="w01")
            w0 = w01[:, 0:EMB]
            w1 = w01[:, EMB : 2 * EMB]

            hk0 = [
                hank_pool.tile([128, c], fin, tag=f"hk{j}", name=f"hk{j}")
                for j, c in enumerate(CHUNKS)
            ]
            hk1 = hank_pool.tile([128, HANK_COLS], fin, tag="hkb1")

            def load_chunk0(j):
                nc.sync.dma_start(
                    hk0[j][:, :],
                    AP(xpad_h, OFFS[j], [[1, 128], [1, CHUNKS[j]]]),
                )

            nc.sync.dma_start(w0, w2_h[:, 0:EMB])
            load_chunk0(0)
            nc.sync.dma_start(w1, w2_h[:, EMB : 2 * EMB])
            load_chunk0(1)
            load_chunk0(2)
            load_chunk0(3)
            # batch-1 Hankel: ONE big DMA, on the sync ring BEHIND the
            # batch-0 chunks -- ring FIFO guarantees it cannot steal engine
            # time from the critical head (the Tile scheduler hoists engine
            # instructions, so putting it on the idle scalar ring would
            # flood the shared engine pool right at kernel start)
            nc.sync.dma_start(
                hk1[:, :], AP(xpad_h, XP_LEN, [[1, 128], [1, HANK_COLS]])
            )

            def hank_slice(b, c):
                """lhsT for column-block c (128 cols starting at 128*c)."""
                lo = 128 * c
                if b == 1:
                    return hk1[:, lo : lo + 128]
                for j in range(len(CHUNKS)):
                    if lo + 128 <= OFFS[j + 1]:
                        off = lo - OFFS[j]
                        return hk0[j][:, off : off + 128]
                raise AssertionError(c)

            qglob = 0
            for b in range(B_PER):
                for g in range(N_SUP):
                    sup = sup_pool.tile([128, SUP_W], fout)
                    for pq in range(PAIRS_PER_SUP):
                        ps = psum_pool.tile([128, 2 * EMB], f32)  # 2 banks
                        for h in range(2):
                            i = (g * PAIRS_PER_SUP + pq) * 2 + h
                            pslice = ps[:, h * EMB : (h + 1) * EMB]
                            nc.tensor.matmul(
                                pslice, hank_slice(b, i), w0,
                                start=True, stop=False,
                            )
                            nc.tensor.matmul(
                                pslice, hank_slice(b, i + 1), w1,
                                start=False, stop=True,
                            )
                        dst = sup[:, pq * 2 * EMB : (pq + 1) * 2 * EMB]
                        # bias is added on the HOST during the mandatory
                        # fp16->fp32 upcast, so evacuation is a single pure
                        # copy alternating ACT / DVE -- each engine runs at
                        # ~65% even in the chip's 2.0 GHz power state
                        # (fused-bias variants sat at ~90%+ and stalled the
                        # PE whenever the clock dropped)
                        if qglob % 2 == 0:
                            nc.scalar.copy(dst, ps[:, :])
                        else:
                            nc.vector.tensor_copy(dst, ps[:, :])
                        qglob += 1
                    # store supertile: plain 2D slice in the partition-major
                    # layout -- one 8 KB contiguous run per partition
                    col0 = g * SUP_W
                    last = b == B_PER - 1 and g == N_SUP - 1
                    if last:
                        # final supertile: four quarter-stores alternating
                        # rings, so each pair's data departs as soon as its
                        # evacuation lands and the last receipt comes early
                        q = SUP_W // 4
                        for v in range(4):
                            eng = nc.sync if v % 2 == 0 else nc.scalar
                            eng.dma_start(
                                out_h[b, :, col0 + v * q : col0 + (v + 1) * q],
                                sup[:, v * q : (v + 1) * q],
                            )
                    else:
                        # most stores ride the sync ring: its trigger queue
                        # is idle, while every scalar-ring trigger costs
                        # ~740ns of ACT time that the evacuation needs.
                        # A third of stores still go to scalar so both
                        # HWDGE rings contribute descriptor throughput.
                        gi = b * N_SUP + g
                        eng = nc.scalar if gi % 3 == 1 else nc.sync
                        eng.dma_start(out_h[b, :, col0 : col0 + SUP_W], sup[:, :])

    nc.finalize()
    return nc


def _get_program():
    key = ("prog", USE_DT, OUT_DT)
    if key not in _CACHE:
        _CACHE[key] = _build_program()
    return _CACHE[key]


def kernel(x: np.ndarray, weight: np.ndarray, bias: np.ndarray) -> np.ndarray:
    global LAST_RESULT
    from concourse.bass_utils import run_bass_kernel_spmd

    x = np.asarray(x, dtype=np.float32)
    weight = np.asarray(weight, dtype=np.float32)
    bias = np.asarray(bias, dtype=np.float32)

    m2 = _build_m2(weight).astype(np.float32)
    xpad = np.zeros((B, XP_LEN), dtype=np.float32)
    xpad[:, PAD : PAD + T] = x
    # pack to the SBUF tile layout: w2[p, h*EMB+e] = M2[128h+p, e]
    w2_in = np.ascontiguousarray(
        m2.reshape(2, 128, EMB).transpose(1, 0, 2).reshape(128, 2 * EMB)
    )

    import ml_dtypes

    np_in = {
        "fp16": np.float16,
        "bf16": ml_dtypes.bfloat16,
        "f32r": np.float32,
        "f32": np.float32,
    }[USE_DT]
    np_out = {"fp16": np.float16, "bf16": ml_dtypes.bfloat16, "f32": np.float32}[OUT_DT]
    if USE_DT == "f32r":
        w2_in = _round_fp22(w2_in)
        xpad = _round_fp22(xpad)
    else:
        w2_in = w2_in.astype(np_in)
        xpad = xpad.astype(np_in)
    nc = _get_program()
    in_maps = [
        {
            "xpad": np.ascontiguousarray(xpad[c * B_PER : (c + 1) * B_PER]),
            "w2": w2_in,
        }
        for c in range(N_CORES)
    ]
    res = run_bass_kernel_spmd(nc, in_maps, list(range(N_CORES)), trace=TRACE)
    LAST_RESULT = res
    # un-permute the partition-major device layout
    # (out_dev[b, p, c*EMB+e] -> out[b, 128c+p, e]) and fuse the bias add
    # into the fp16 -> fp32 upcast
    outs = []
    for c in range(N_CORES):
        od = np.asarray(res.results[c]["out"])  # [B_PER, 128, N_TILES*EMB]
        od = od.reshape(B_PER, 128, N_TILES, EMB).transpose(0, 2, 1, 3)
        outs.append(od.reshape(B_PER, T, EMB))
    out = np.concatenate(outs, axis=0).astype(np.float32)
    out += bias[None, None, :]
    return np.ascontiguousarray(out)
